# revision 28
# baseline (speedup 1.0000x reference)
"""2D Haar DWT (single level) on Trainium2, 8 NeuronCores, pure data parallel.

Math: with Haar filters + symmetric pad + odd-phase downsample, the DWT
reduces to per-2x2-block butterflies over the input image x:
  ll = 0.5*(x00 + x01 + x10 + x11)   (top-left quadrant of output)
  lh = 0.5*(x00 + x01 - x10 - x11)   (bottom-left)
  hl = 0.5*(x00 - x01 + x10 - x11)   (top-right)
  hh = 0.5*(x00 - x01 - x10 + x11)   (bottom-right)

Pipeline of units per core (8 images): half-image, half-image, then image
pairs, then two half-images — tapered so the first out-DMA starts as
early as possible (more load/store overlap on the HBM stream) and the
tail chain is short.  In-DMAs on the SP HWDGE ring; out-DMAs on the ACT
ring (separate FIFO rings avoid head-of-line blocking).

Width-pass pair SUMS via one DVE tensor_reduce reading X sequentially
(DVE pays ~3x for strided reads — avoid); width-pass pair DIFFS on
GpSimd with strided reads (software engine, stride-insensitive).  Both
write bf16 T (rel-err budget 2e-2; bf16 keeps the height pass in DVE 2x
mode).  Height pass: wide 2-level-AP bf16 adds/subs on DVE.  ACT
ACTIVATEs apply the 0.5 scale AND cast bf16->f32, then out-DMAs stream
per half.

Pair units: X[128, 2048*n], partition p holds rows 4p..4p+3 per image;
per image Y[p, c*1024 + q*512 + w] = out[c*256 + 2p + q, w].
Half units (rows h*256..h*256+255): partition p holds rows h*256+2p,
h*256+2p+1 -> output rows h*128+p (ll|hl) and 256+h*128+p (lh|hh).
"""

import numpy as np

import concourse.mybir as mybir
from concourse import bacc, tile
from concourse.bass_utils import run_bass_kernel_spmd

N_CORES = 8
BATCH = 64
B_PER = BATCH // N_CORES  # 8 images per core
H = W = 512

_nc_cache = None


def build_bass():
    f32 = mybir.dt.float32
    bf16 = mybir.dt.bfloat16
    nc = bacc.Bacc(
        "TRN2", target_bir_lowering=False, debug=False, num_devices=N_CORES
    )
    inp = nc.dram_tensor("inputs", [B_PER, H, W], f32, kind="ExternalInput").ap()
    out = nc.dram_tensor("out", [B_PER, H, W], f32, kind="ExternalOutput").ap()

    with tile.TileContext(nc) as tc:
        pool_cm = tc.tile_pool(name="p", bufs=3)
        pool = pool_cm.__enter__()

        def pair_unit(i, n):
            """n consecutive full images starting at image i."""
            F = 2048 * n
            X = pool.tile([128, F], f32, tag="X", bufs=4)
            nc.sync.dma_start(
                out=X[:],
                in_=inp[i : i + n].rearrange("j (p r) w -> p j r w", p=128),
            )
            # width pass: per image j, T[:, j*2048+0:1024] = pair sums
            # (r-blocks of 256), T[:, j*2048+1024:2048] = diffs
            T = pool.tile([128, F], bf16, tag="T")
            with nc.allow_low_precision(reason="bf16 DWT intermediates"):
                nc.vector.tensor_reduce(
                    out=T[:].rearrange("p (j d x) -> p j d x", j=n, d=2)[:, :, 0, :],
                    in_=X[:].rearrange("p (j r k t) -> p (j r) k t", j=n, r=4, t=2),
                    axis=mybir.AxisListType.X,
                    op=mybir.AluOpType.add,
                )
            for j in range(n):
                for r in range(4):
                    o = j * 2048
                    nc.gpsimd.tensor_sub(
                        out=T[:, o + 1024 + r * 256 : o + 1024 + (r + 1) * 256],
                        in0=X[:, o + r * 512 : o + (r + 1) * 512 : 2],
                        in1=X[:, o + r * 512 + 1 : o + (r + 1) * 512 : 2],
                    )
            # height pass (bf16 2x on DVE), unit-wide 2-level ops
            Yb = pool.tile([128, F], bf16, tag="Yb", bufs=4)
            Tv = T[:].rearrange("p (j d q r k) -> p j d q r k", j=n, d=2, q=2, r=2)
            Yv = Yb[:].rearrange("p (j c q h k) -> p j c q h k", j=n, c=2, q=2, h=2)
            nc.vector.tensor_add(
                out=Yv[:, :, 0, :, 0, :], in0=Tv[:, :, 0, :, 0, :], in1=Tv[:, :, 0, :, 1, :]
            )
            nc.vector.tensor_sub(
                out=Yv[:, :, 1, :, 0, :], in0=Tv[:, :, 0, :, 0, :], in1=Tv[:, :, 0, :, 1, :]
            )
            nc.vector.tensor_add(
                out=Yv[:, :, 0, :, 1, :], in0=Tv[:, :, 1, :, 0, :], in1=Tv[:, :, 1, :, 1, :]
            )
            nc.vector.tensor_sub(
                out=Yv[:, :, 1, :, 1, :], in0=Tv[:, :, 1, :, 0, :], in1=Tv[:, :, 1, :, 1, :]
            )
            # fused 0.5 scale + bf16->f32 cast on ACT, then 512 KiB out-DMAs
            Y = pool.tile([128, F], f32, tag="Y", bufs=4)
            for j in range(n):
                for c in range(2):
                    sl = slice(j * 2048 + c * 1024, j * 2048 + (c + 1) * 1024)
                    nc.scalar.mul(Y[:, sl], Yb[:, sl], 0.5)
                    nc.scalar.dma_start(
                        out=out[i + j, c * 256 : (c + 1) * 256].rearrange(
                            "(p q) w -> p q w", q=2
                        ),
                        in_=Y[:, sl],
                    )

        def half_unit(i, h):
            """Half (rows h*256..h*256+255) of image i; partition p holds
            rows h*256+2p, h*256+2p+1 -> output rows h*128+p, 256+h*128+p."""
            X = pool.tile([128, 1024], f32, tag="X", bufs=4)
            nc.sync.dma_start(
                out=X[:],
                in_=inp[i, h * 256 : (h + 1) * 256].rearrange(
                    "(p r) w -> p r w", p=128
                ),
            )
            # T: [s_r0 s_r1 | d_r0 d_r1] blocks of 256
            T = pool.tile([128, 1024], bf16, tag="T")
            with nc.allow_low_precision(reason="bf16 DWT intermediates"):
                nc.vector.tensor_reduce(
                    out=T[:, 0:512],
                    in_=X[:].rearrange("p (r k t) -> p r k t", r=2, t=2),
                    axis=mybir.AxisListType.X,
                    op=mybir.AluOpType.add,
                )
            for r in range(2):
                nc.gpsimd.tensor_sub(
                    out=T[:, 512 + r * 256 : 512 + (r + 1) * 256],
                    in0=X[:, r * 512 : (r + 1) * 512 : 2],
                    in1=X[:, r * 512 + 1 : (r + 1) * 512 : 2],
                )
            # height pass: m = p only -> Yb = [ll|hl|lh|hh] blocks of 256
            Yb = pool.tile([128, 1024], bf16, tag="Yb", bufs=4)
            nc.vector.tensor_add(out=Yb[:, 0:256], in0=T[:, 0:256], in1=T[:, 256:512])
            nc.vector.tensor_add(
                out=Yb[:, 256:512], in0=T[:, 512:768], in1=T[:, 768:1024]
            )
            nc.vector.tensor_sub(
                out=Yb[:, 512:768], in0=T[:, 0:256], in1=T[:, 256:512]
            )
            nc.vector.tensor_sub(
                out=Yb[:, 768:1024], in0=T[:, 512:768], in1=T[:, 768:1024]
            )
            Y = pool.tile([128, 1024], f32, tag="Y", bufs=4)
            for c in range(2):  # c=0: out row h*128+p = [ll|hl]; c=1: +256
                sl = slice(c * 512, (c + 1) * 512)
                nc.scalar.mul(Y[:, sl], Yb[:, sl], 0.5)
                nc.scalar.dma_start(
                    out=out[i, c * 256 + h * 128 : c * 256 + h * 128 + 128],
                    in_=Y[:, sl],
                )

        half_unit(0, 0)
        half_unit(0, 1)
        pair_unit(1, 2)
        pair_unit(3, 2)
        pair_unit(5, 2)
        half_unit(7, 0)
        half_unit(7, 1)

        pool_cm.__exit__(None, None, None)
    # close TileContext via with-block semantics above

    nc.compile()
    return nc


def kernel(**inputs):
    global _nc_cache
    x = np.ascontiguousarray(
        np.asarray(inputs["inputs"], dtype=np.float32).reshape(BATCH, H, W)
    )
    if _nc_cache is None:
        _nc_cache = build_bass()
    nc = _nc_cache
    in_maps = [
        {"inputs": x[i * B_PER : (i + 1) * B_PER]} for i in range(N_CORES)
    ]
    res = run_bass_kernel_spmd(nc, in_maps, core_ids=list(range(N_CORES))).results
    out = np.concatenate([res[i]["out"] for i in range(N_CORES)], axis=0)
    return out.reshape(BATCH, H, W, 1)
